# revision 16
# baseline (speedup 1.0000x reference)
"""GAT layer (nn_GAT_49589692400146) on 8 TRN2 NeuronCores.

Row-shard over nodes, SPMD (identical per-core module, no collectives).

v3 design (baseline v1 ~100.7us):
  - Host folds the per-column z1 term and the additive mask into one fp16
    array: maskp[j, i] = z1_i + (adj[i,j] ? 0 : -448). The device logit
    tile is then maskp + z2_j, with z2 folded into the activation bias,
    so the only full-size elementwise passes are LeakyReLU and exp.
  - LeakyReLU split across engines by column range (tunable): ACT Prelu
    (bias=z2), DVE 3-op max(t+z2, 0.2(t+z2)), Pool scalar_tensor_tensor.
  - exp on ACT writes attention weights pm directly as fp8e5.
  - P2 (att @ Wh) via fp8 DoubleRow matmuls (K=256/instr, 0.5 cyc/col)
    with split-precision rhs: A = e4m3(16*Wh), B = e4m3(16*Wh - A);
    h = (accA + accB) / den, den from A's 16.0-ones column. (Validated
    end-to-end in fp8 emulation: rel err ~2.3e-3.)
  - Wh eviction: A on ACT(Copy,scale=16)/DVE alternating, B residual on
    DVE (Pool cannot read PSUM).
  - Emission staggers P2(g-1) after P1(g)/elementwise(g) so the PE queue
    never waits on the exp of the group it accumulates.
Host: out = concat(hc) + (sum_c s_c) @ fcW_bot + fcb.
"""

import numpy as np

import concourse.bacc as bacc
import concourse.tile as tile
import concourse.mybir as mybir
from concourse import bass_utils

F32 = mybir.dt.float32
F16 = mybir.dt.float16
F8E4 = mybir.dt.float8e4
F8E5 = mybir.dt.float8e5
ALU = mybir.AluOpType
AF = mybir.ActivationFunctionType
MPM = mybir.MatmulPerfMode

NCORES = 8
N_FULL = 6144
NF = 512
NH = 256
ALPHA = 0.2
EXP_SHIFT = 8.0
MASK_NEG = -448.0
ASCALE = 16.0

# --- tunables (column counts are scaled by R/768 at build time) ---
CA = 256          # prelu columns on ACT (bias=z2)
CD = 128          # prelu columns on DVE (3-op)
A_EVICT = ["act", "act", "act", "dve"]   # cycle for A eviction engine
GS = 8            # j-tiles per group (must be even)

_BUILD_CACHE = {}


def _build(NN, R):
    P = 128
    T = NN // P
    NPAIR = T // 2
    IC = R // P
    KT = NF // P
    KH = NH // P
    assert T % GS == 0 and GS % 2 == 0 and R % P == 0
    NG = T // GS
    PPG = GS // 2
    ca = int(round(R * CA / 768.0))
    cd = int(round(R * CD / 768.0))
    cp = R - ca - cd

    nc = bacc.Bacc("TRN2", target_bir_lowering=False, debug=False)

    xTp = nc.dram_tensor("xTp", [NF, NN], F16, kind="ExternalInput").ap()
    maskp = nc.dram_tensor("maskp", [NN, R], F16, kind="ExternalInput").ap()
    w_in = nc.dram_tensor("w_in", [NF, NH], F16, kind="ExternalInput").ap()
    wt_in = nc.dram_tensor("wt_in", [NH, NF], F16, kind="ExternalInput").ap()
    a_in = nc.dram_tensor("a_in", [P, 2 * KH], F16, kind="ExternalInput").ap()
    fcw_in = nc.dram_tensor("fcw_in", [1, NH], F16, kind="ExternalInput").ap()

    hc_out = nc.dram_tensor("hc_out", [R, 1], F32, kind="ExternalOutput").ap()
    sc_out = nc.dram_tensor("sc_out", [1, NH], F32, kind="ExternalOutput").ap()

    xTp_r = xTp.rearrange("(k p) n -> k p n", p=P)
    maskp_r = maskp.rearrange("(t p) r -> t p r", p=P)
    w_r = w_in.rearrange("(k p) h -> k p h", p=P)
    wt_r = wt_in.rearrange("(k p) f -> k p f", p=P)

    with tile.TileContext(nc) as tc:
        import contextlib

        with contextlib.ExitStack() as ctx:
            pXT = ctx.enter_context(tc.tile_pool(name="pXT", bufs=1))
            pCst = ctx.enter_context(tc.tile_pool(name="pCst", bufs=1))
            pAB = ctx.enter_context(tc.tile_pool(name="pAB", bufs=2))
            pP = ctx.enter_context(tc.tile_pool(name="pP", bufs=2))
            pM = ctx.enter_context(tc.tile_pool(name="pM", bufs=2))
            pS = ctx.enter_context(tc.tile_pool(name="pS", bufs=6))
            psW = ctx.enter_context(tc.tile_pool(name="psW", bufs=2, space="PSUM"))
            psA = ctx.enter_context(tc.tile_pool(name="psA", bufs=1, space="PSUM"))

            # ---- constants / weights ----
            waug = []
            for k in range(KT):
                wk = pCst.tile([P, NH + 1], F16, tag=f"waug{k}", name=f"wk{k}")
                nc.sync.dma_start(wk[:, 1:NH + 1], w_r[k])
                waug.append(wk)

            wt_sb = []
            for k in range(KH):
                wtk = pCst.tile([P, NF], F16, tag=f"wt{k}", name=f"wtk{k}")
                nc.sync.dma_start(wtk[:], wt_r[k])
                wt_sb.append(wtk)

            a_sb = pCst.tile([P, 2 * KH], F16, tag="a_sb")
            nc.sync.dma_start(a_sb[:], a_in)

            fcwb = pCst.tile([P, NH], F16, tag="fcwb")
            nc.gpsimd.dma_start(fcwb[:], fcw_in.partition_broadcast(P))

            ones_col = pCst.tile([P, 1], F16, tag="ones_col")
            nc.gpsimd.memset(ones_col[:], 1.0)
            shift_col = pCst.tile([P, 1], F32, tag="shift_col")
            nc.gpsimd.memset(shift_col[:], -EXP_SHIFT)
            sc16 = pCst.tile([P, 1], F32, tag="sc16")
            nc.gpsimd.memset(sc16[:], ASCALE)

            xt = []
            for k in range(KT):
                xk = pXT.tile([P, NN], F16, tag=f"xt{k}", name=f"xk{k}")
                xt.append(xk)
            for p0 in range(0, NN, R):
                for k in range(KT):
                    nc.sync.dma_start(xt[k][:, p0:p0 + R], xTp_r[k][:, p0:p0 + R])

            # ---- P0: Wa2 column per feat chunk (waug col 0) ----
            for mc in range(KT):
                pwa = psW.tile([P, 2], F32, tag="work", name=f"pwa{mc}")
                for k in range(KH):
                    nc.tensor.matmul(
                        pwa[:],
                        wt_sb[k][:, mc * P:(mc + 1) * P],
                        a_sb[:, 2 * k:2 * k + 2],
                        start=(k == 0),
                        stop=(k == KH - 1),
                    )
                nc.vector.tensor_copy(waug[mc][:, 0:1], pwa[:, 0:1])

            # ---- PSUM accumulators: one 512-col bank per i-chunk,
            # [A(256) | den | B(255)] so 12 groups fit 6 banks ----
            NB = NH - 1
            accT = [psA.tile([P, 2 * NH], F32, tag=f"acc{i}", name=f"acc{i}")
                    for i in range(IC)]
            accA = [t[:, 0:NH + 1] for t in accT]
            accB = [t[:, NH + 1:2 * NH] for t in accT]
            NBW = NH - 1

            z2g = [pCst.tile([P, GS], F32, tag=f"z2g{g}", name=f"z2g{g}")
                   for g in range(NG)]

            def emit_group_p1(g):
                ABpairs = []
                for pp in range(PPG):
                    Ap = pAB.tile([P, 2, 2 * NH], F8E4, tag=f"AB{pp}",
                                  name=f"AB{g}_{pp}")
                    nc.gpsimd.memset(Ap[:, :, NH:NH + 1], ASCALE)
                    ABpairs.append(Ap)
                for t in range(GS):
                    jt = g * GS + t
                    pp, s = t // 2, t % 2
                    pc = psW.tile([P, NH + 1], F32, tag="work", name=f"pc{jt}")
                    for k in range(KT):
                        nc.tensor.matmul(
                            pc[:],
                            xt[k][:, jt * P:(jt + 1) * P],
                            waug[k][:],
                            start=(k == 0),
                            stop=(k == KT - 1),
                        )
                    nc.vector.tensor_copy(z2g[g][:, t:t + 1], pc[:, 0:1])
                    a_slot = ABpairs[pp][:, s, 0:NH]
                    b_slot = ABpairs[pp][:, s, NH + 1:2 * NH]
                    if A_EVICT[jt % len(A_EVICT)] == "act":
                        nc.scalar.activation(a_slot, pc[:, 1:NH + 1], AF.Copy,
                                             scale=sc16[:])
                    else:
                        nc.vector.tensor_scalar_mul(a_slot, pc[:, 1:NH + 1], ASCALE)
                    nc.vector.scalar_tensor_tensor(
                        b_slot, pc[:, 1:NB + 1], ASCALE,
                        ABpairs[pp][:, s, 0:NB],
                        ALU.mult, ALU.subtract)
                return ABpairs

            def emit_group_elem(g, mk):
                pms = []
                for pp in range(PPG):
                    pmp = pP.tile([P, 2, R], F8E5, tag=f"pm{pp}", name=f"pm{g}_{pp}")
                    pms.append(pmp)
                for t in range(GS):
                    pp, s = t // 2, t % 2
                    tm = mk[:, t * R:(t + 1) * R]
                    z2c = z2g[g][:, t:t + 1]
                    c0 = 0
                    if ca:
                        nc.scalar.activation(
                            tm[:, c0:c0 + ca], tm[:, c0:c0 + ca], AF.Prelu,
                            bias=z2c, alpha=ALPHA)
                        c0 += ca
                    for width, t2eng in ((cd, "dve"), (cp, "pool")):
                        if not width:
                            continue
                        sl = slice(c0, c0 + width)
                        t2 = pS.tile([P, width], F16, tag=f"t2{t2eng}",
                                     name=f"t2_{t2eng}_{g}_{t}")
                        eng = nc.vector if t2eng == "dve" else nc.gpsimd
                        eng.tensor_scalar(
                            t2[:], tm[:, sl], z2c, ALPHA, op0=ALU.add, op1=ALU.mult)
                        nc.vector.tensor_scalar_add(tm[:, sl], tm[:, sl], z2c)
                        nc.vector.tensor_tensor(tm[:, sl], tm[:, sl], t2[:], op=ALU.max)
                        c0 += width
                    nc.scalar.activation(pms[pp][:, s, :], tm[:], AF.Exp,
                                         bias=shift_col[:])
                return pms

            def emit_group_p2(g, pms, ABpairs):
                for pp in range(PPG):
                    pi = g * PPG + pp
                    pm3 = pms[pp]
                    for i in range(IC):
                        lhs = pm3[:, :, i * P:(i + 1) * P]
                        nc.tensor.matmul(
                            accT[i][:], lhs, ABpairs[pp][:, :, :],
                            start=(pi == 0), stop=(pi == NPAIR - 1),
                            perf_mode=MPM.DoubleRow)

            # ---- staggered main loop ----
            prev = None
            for g in range(NG):
                mk = pM.tile([P, GS * R], F16, tag="mask", name=f"mk{g}")
                for t in range(GS):
                    nc.sync.dma_start(mk[:, t * R:(t + 1) * R], maskp_r[g * GS + t])
                ABpairs = emit_group_p1(g)
                pms = emit_group_elem(g, mk)
                if prev is not None:
                    emit_group_p2(*prev)
                prev = (g, pms, ABpairs)
            emit_group_p2(*prev)

            # ---- P3: h = (accA + accB)/den, ELU, outputs ----
            hc_sb = pCst.tile([P, IC], F32, tag="hc_sb")
            nc.gpsimd.memset(hc_sb[:], 0.0)
            sacc = psW.tile([1, NH], F32, tag="work", name="sacc")
            s_sb = pCst.tile([1, NH], F32, tag="s_sb")
            for i in range(IC):
                bsum = pS.tile([P, NH], F32, tag="bsum", name=f"bsum{i}")
                nc.vector.tensor_copy(bsum[:, 0:NB], accB[i][:])
                nc.gpsimd.memset(bsum[:, NB:NH], 0.0)
                nsum = pS.tile([P, NH], F32, tag="nsum", name=f"nsum{i}")
                nc.vector.tensor_tensor(nsum[:], accA[i][:, 0:NH], bsum[:],
                                        op=ALU.add)
                rec = pS.tile([P, 1], F32, tag="rec", name=f"rec{i}")
                nc.vector.reciprocal(rec[:], accA[i][:, NH:NH + 1])
                h = pS.tile([P, NH], F32, tag="h", name=f"h{i}")
                nc.vector.tensor_scalar_mul(h[:], nsum[:], rec[:])
                ex = pS.tile([P, NH], F32, tag="ex", name=f"ex{i}")
                nc.scalar.activation(ex[:], h[:], AF.Exp)
                rl = pS.tile([P, NH], F32, tag="rl", name=f"rl{i}")
                nc.vector.tensor_scalar_max(rl[:], h[:], 0.0)
                he = pS.tile([P, NH], F16, tag="he", name=f"he{i}")
                nc.vector.scalar_tensor_tensor(
                    he[:], ex[:], -1.0, rl[:], ALU.add, ALU.min)
                nc.tensor.matmul(
                    sacc[:], ones_col[:], he[:],
                    start=(i == 0), stop=(i == IC - 1),
                )
                hw = pS.tile([P, NH], F16, tag="hw", name=f"hw{i}")
                nc.vector.scalar_tensor_tensor(
                    hw[:], he[:], 1.0, fcwb[:],
                    ALU.mult, ALU.mult, accum_out=hc_sb[:, i:i + 1])

            nc.vector.tensor_copy(s_sb[:], sacc[:])
            nc.sync.dma_start(sc_out, s_sb[:])
            nc.sync.dma_start(
                hc_out.rearrange("(a p) o -> p (a o)", p=P), hc_sb[:])

    nc.compile()
    return nc


def _get_module(NN, R):
    key = (NN, R)
    if key not in _BUILD_CACHE:
        _BUILD_CACHE[key] = _build(NN, R)
    return _BUILD_CACHE[key]


def _make_in_maps(x, adj, W, a, fcW, n_cores=NCORES):
    NN = x.shape[0]
    R = NN // n_cores
    P = 128
    KH = NH // P

    xT = np.ascontiguousarray(x.T).astype(np.float16)
    W16 = W.astype(np.float16)
    WT16 = np.ascontiguousarray(W16.T)
    a16 = a.astype(np.float16)[:, 0]
    a_t = np.zeros((P, 2 * KH), np.float16)
    for k in range(KH):
        a_t[:, 2 * k] = a16[NH + k * P:NH + (k + 1) * P]      # a2 chunk k
        a_t[:, 2 * k + 1] = a16[k * P:(k + 1) * P]            # a1 chunk k
    fcw_row = fcW[:NH, 0].astype(np.float16)[None, :]

    # z1_i = x_i . (W @ a1), fp16-consistent with the device's fp16 inputs
    wa1 = (W16.astype(np.float32) @ a16[:NH].astype(np.float32)).astype(
        np.float16).astype(np.float32)
    z1 = (xT.astype(np.float32).T @ wa1).astype(np.float16)   # [NN]
    maskT = np.where(adj > 0, np.float16(0.0), np.float16(MASK_NEG)).T
    mz = (maskT.astype(np.float32) + z1[None, :].astype(np.float32)).astype(
        np.float16)                                           # [j, i] full

    in_maps = []
    for c in range(n_cores):
        r0, r1 = c * R, (c + 1) * R
        xTp = np.concatenate([xT[:, r0:r1], xT[:, :r0], xT[:, r1:]], axis=1)
        mT = mz[:, r0:r1]
        maskp = np.concatenate([mT[r0:r1], mT[:r0], mT[r1:]], axis=0)
        in_maps.append({
            "xTp": np.ascontiguousarray(xTp),
            "maskp": np.ascontiguousarray(maskp),
            "w_in": W16,
            "wt_in": WT16,
            "a_in": a_t,
            "fcw_in": fcw_row,
        })
    return in_maps


def _run_sharded(x, adj, W, a, fcW, fcb, n_cores=NCORES, **run_kwargs):
    NN = x.shape[0]
    R = NN // n_cores
    nc = _get_module(NN, R)
    in_maps = _make_in_maps(x, adj, W, a, fcW, n_cores)

    res = bass_utils.run_bass_kernel_spmd(
        nc, in_maps, core_ids=list(range(n_cores)), **run_kwargs
    )

    hc = np.concatenate([res.results[c]["hc_out"] for c in range(n_cores)], axis=0)
    s = np.sum([res.results[c]["sc_out"] for c in range(n_cores)], axis=0)[0]
    const = s.astype(np.float64) @ fcW[NH:, 0].astype(np.float64) + float(fcb[0])
    out = hc + np.float32(const)
    return out.astype(np.float32), res


def kernel(x, adj, W, a, fcW, fcb):
    out, _ = _run_sharded(
        np.asarray(x), np.asarray(adj), np.asarray(W),
        np.asarray(a), np.asarray(fcW), np.asarray(fcb),
    )
    return out
